# revision 2
# baseline (speedup 1.0000x reference)
"""Trainium2 Bass kernel for the CellLoss problem.

loss = mean_i [ 1/(x[i, l_i] + 0.1) + sum_j x[i,j] * (x[i,j] > x[i, l_i]) ]
with x: [131072, 256] f32, l: [131072] int labels in [0, 256).

Pure data parallel across 8 NeuronCores (16384 rows each). Per core,
partition p owns rows [p*128, (p+1)*128) of the shard; tile t is the
[128, 256] block of row p*128+t per partition.

Per tile:
  gather (DVE): g[p] = sum_j (iota==l_p)*x via one fused
      scalar_tensor_tensor (stt) with a per-row sum accumulator.
  margin, engine variants cycled by PATTERN:
   "D": one more DVE stt, (x is_gt g) mult x with row-sum accumulator.
   "G": same stt on the GpSimd engine.
   "A": two scalar-engine activation passes, each with a row-sum
      accumulator (no matmuls): SR = sum relu(x - g), SS = sum sign(g-x).
      With sign(0)=0 only at the label column, the count of strictly
      greater elements is N+ = (255 - SS)/2, so
      margin = SR + g*(255 - SS)/2, assembled per-row in the tail.
Tail: inv = 1/(g+0.1); per-row totals; partition sum via ones-matmul;
one f32 partial per core; the host sums the 8 partials and divides by B.

All arithmetic on x stays f32 (exact gather for the inv term); the ACT
scratch outputs are written as bf16 only to cut SBUF write bandwidth —
their accumulators are fp32-internal.

This walrus accepts one sync wait per instruction; Tile can emit
several. _split_multi_waits() hoists extras onto Drain carriers.
"""

import numpy as np
from contextlib import ExitStack

import concourse.bass as bass
import concourse.mybir as mybir
import concourse.tile as tile
from concourse.bass_utils import run_bass_kernel_spmd

F32 = mybir.dt.float32
BF16 = mybir.dt.bfloat16

B, C = 131072, 256
N_CORES = 8
B_LOCAL = B // N_CORES          # 16384
P = 128
N_TILES = B_LOCAL // P          # 128
TILES_PER_DMA = 16              # [128, 4096] f32 = 2 MiB per DMA
N_CHUNKS = N_TILES // TILES_PER_DMA

# margin engine per tile within each 16-tile chunk:
#   "D" DVE stt / "A" scalar-engine relu+sign / "G" gpsimd stt
# "A" tiles must form the cycle tail (tail assembly uses a strided view).
PATTERN = list("DDDDDGGGAAAAAAAA")

_NC_CACHE = {}
LAST_RESULTS = None
SPLIT_WAITS = True   # off for CoreSim (its event loop rejects bare Drains)
TRACE = False
TRACE_KW = {}


def _split_multi_waits(nc):
    for f in nc.m.functions:
        for blk in f.blocks:
            insts = list(blk.instructions)
            out = []
            changed = False
            for inst in insts:
                si = inst.sync_info
                if si is not None and si.on_wait is not None and len(si.on_wait) > 1:
                    waits = list(si.on_wait)
                    for w in waits[:-1]:
                        d = mybir.InstDrain(
                            name=nc.get_next_instruction_name(),
                            ins=[], outs=[], bass_is_fusable=False)
                        d.engine = inst.engine
                        d.sync_info = mybir.SyncInfo(on_wait=[w], on_update=[])
                        out.append(d)
                    inst.sync_info = mybir.SyncInfo(
                        on_wait=[waits[-1]], on_update=list(si.on_update or []))
                    changed = True
                out.append(inst)
            if changed:
                blk.instructions = out


def _assignment():
    assert N_TILES % len(PATTERN) == 0
    return [PATTERN[t % len(PATTERN)] for t in range(N_TILES)]


def build_nc():
    key = (tuple(_assignment()), SPLIT_WAITS)
    if key in _NC_CACHE:
        return _NC_CACHE[key]

    assign = _assignment()
    a_tiles = [t for t, c in enumerate(assign) if c == "A"]
    acol = {t: i for i, t in enumerate(a_tiles)}
    n_a = len(a_tiles)
    L = len(PATTERN)
    nA_cyc = sum(1 for c in PATTERN if c == "A")
    a0 = L - nA_cyc
    if n_a:
        assert all(c == "A" for c in PATTERN[a0:])

    nc = bass.Bass()
    x = nc.declare_dram_parameter("x", [B_LOCAL, C], F32, isOutput=False)
    lbl = nc.declare_dram_parameter("lbl", [P, N_TILES], F32, isOutput=False)
    out = nc.declare_dram_parameter("out", [1, 1], F32, isOutput=True)

    xv = x.rearrange("(p t) c -> p (t c)", p=P, t=N_TILES)

    with tile.TileContext(nc) as tc, ExitStack() as ctx:
        singles = ctx.enter_context(tc.tile_pool(name="singles", bufs=1))
        xpool = ctx.enter_context(tc.tile_pool(name="x", bufs=3))
        scr = ctx.enter_context(tc.tile_pool(name="scr", bufs=4))
        psum = ctx.enter_context(tc.tile_pool(name="psum", bufs=1, space="PSUM"))

        lbl_sb = singles.tile([P, N_TILES], F32)
        nc.sync.dma_start(lbl_sb[:], lbl[:])

        iota_i = singles.tile([P, C], mybir.dt.int32)
        nc.gpsimd.iota(iota_i[:], pattern=[[1, C]], base=0, channel_multiplier=0)
        iota_f = singles.tile([P, C], F32)
        nc.vector.tensor_copy(iota_f[:], iota_i[:])

        ones = singles.tile([P, 1], F32)
        nc.vector.memset(ones[:], 1.0)

        G = singles.tile([P, N_TILES], F32)
        NG = singles.tile([P, N_TILES], F32)
        M = singles.tile([P, N_TILES], F32)      # D/G margins; A cols = 0
        if n_a:
            nc.vector.memset(M[:], 0.0)
            SR = singles.tile([P, n_a], F32)     # sum relu(x-g)
            SS = singles.tile([P, n_a], F32)     # sum sign(g-x)
            # dedicated ACT scratch, reused every A tile (same-engine WAW)
            act_scr = singles.tile([P, 2 * C], BF16)

        for chunk in range(N_CHUNKS):
            xw = xpool.tile([P, TILES_PER_DMA * C], F32, name="xw")
            nc.sync.dma_start(
                xw[:],
                xv[:, chunk * TILES_PER_DMA * C:(chunk + 1) * TILES_PER_DMA * C])
            t_lo = chunk * TILES_PER_DMA
            t_hi = t_lo + TILES_PER_DMA
            for kk in range(TILES_PER_DMA):
                t = t_lo + kk
                xb = xw[:, kk * C:(kk + 1) * C]
                lc = lbl_sb[:, t:t + 1]
                gc = G[:, t:t + 1]
                sel = scr.tile([P, C], F32, tag="sel", name="sel")
                nc.vector.scalar_tensor_tensor(
                    out=sel[:], in0=iota_f[:], scalar=lc, in1=xb,
                    op0=mybir.AluOpType.is_equal, op1=mybir.AluOpType.mult,
                    accum_out=gc)
            # batch-negate the chunk's gathered values (relu bias = -g)
            if n_a:
                nc.vector.tensor_scalar_mul(
                    NG[:, t_lo:t_hi], G[:, t_lo:t_hi], -1.0)
            for kk in range(TILES_PER_DMA):
                t = t_lo + kk
                xb = xw[:, kk * C:(kk + 1) * C]
                gc = G[:, t:t + 1]
                if assign[t] == "D" or assign[t] == "G":
                    eng = nc.vector if assign[t] == "D" else nc.gpsimd
                    mp = scr.tile([P, C], F32, tag="mp", name="mp")
                    eng.scalar_tensor_tensor(
                        out=mp[:], in0=xb, scalar=gc, in1=xb,
                        op0=mybir.AluOpType.is_gt, op1=mybir.AluOpType.mult,
                        accum_out=M[:, t:t + 1])
                else:  # "A"
                    j = acol[t]
                    nc.scalar.activation(
                        act_scr[:, 0:C], xb, mybir.ActivationFunctionType.Relu,
                        bias=NG[:, t:t + 1], scale=1.0,
                        accum_out=SR[:, j:j + 1])
                    nc.scalar.activation(
                        act_scr[:, C:2 * C], xb,
                        mybir.ActivationFunctionType.Sign,
                        bias=gc, scale=-1.0,
                        accum_out=SS[:, j:j + 1])

        # ---- tail ------------------------------------------------------
        tmp = scr.tile([P, N_TILES], F32, tag="tail", name="tmp")
        nc.vector.tensor_scalar_add(tmp[:], G[:], 0.1)
        inv = scr.tile([P, N_TILES], F32, tag="tail2", name="inv")
        nc.vector.reciprocal(inv[:], tmp[:])
        tot = scr.tile([P, N_TILES], F32, tag="tail3", name="tot")
        nc.vector.tensor_tensor(out=tot[:], in0=inv[:], in1=M[:],
                                op=mybir.AluOpType.add)
        rows = singles.tile([P, 1], F32)
        nc.vector.tensor_reduce(rows[:], tot[:], axis=mybir.AxisListType.X,
                                op=mybir.AluOpType.add)
        if n_a:
            # margin_A = SR + g * (255 - SS)/2, per A column
            g_a = G.rearrange("p (u k) -> p u k", k=L)[:, :, a0:]
            cnt = singles.tile([P, n_a], F32)
            nc.vector.tensor_scalar(out=cnt[:], in0=SS[:],
                                    scalar1=-0.5, scalar2=127.5,
                                    op0=mybir.AluOpType.mult,
                                    op1=mybir.AluOpType.add)
            cnt_v = cnt.rearrange("p (u k) -> p u k", k=nA_cyc)
            gcnt = singles.tile([P, n_a], F32)
            gcnt_v = gcnt.rearrange("p (u k) -> p u k", k=nA_cyc)
            nc.vector.tensor_tensor(out=gcnt_v, in0=cnt_v, in1=g_a,
                                    op=mybir.AluOpType.mult)
            ma = singles.tile([P, n_a], F32)
            nc.vector.tensor_tensor(out=ma[:], in0=gcnt[:], in1=SR[:],
                                    op=mybir.AluOpType.add)
            rows_a = singles.tile([P, 1], F32)
            nc.vector.tensor_reduce(rows_a[:], ma[:],
                                    axis=mybir.AxisListType.X,
                                    op=mybir.AluOpType.add)
            rows2 = singles.tile([P, 1], F32)
            nc.vector.tensor_tensor(out=rows2[:], in0=rows[:], in1=rows_a[:],
                                    op=mybir.AluOpType.add)
            rows = rows2

        ps_fin = psum.tile([P, 8], F32, tag="fin")
        nc.tensor.matmul(ps_fin[:1, :1], ones[:], rows[:])

        fin = singles.tile([1, 1], F32)
        nc.vector.tensor_copy(fin[:], ps_fin[:1, :1])
        nc.sync.dma_start(out[:], fin[:])

    if SPLIT_WAITS:
        _split_multi_waits(nc)
    _NC_CACHE[key] = nc
    return nc


def _prep_inputs(rna_cell_out, rna_cell_label):
    x = np.ascontiguousarray(np.asarray(rna_cell_out, dtype=np.float32))
    l = np.asarray(rna_cell_label).astype(np.int64)
    assert x.shape == (B, C) and l.shape == (B,)
    in_maps = []
    for i in range(N_CORES):
        xs = x[i * B_LOCAL:(i + 1) * B_LOCAL]
        ls = l[i * B_LOCAL:(i + 1) * B_LOCAL]
        lbl = ls.reshape(P, N_TILES).astype(np.float32)
        in_maps.append({"x": xs, "lbl": np.ascontiguousarray(lbl)})
    return in_maps


def kernel(rna_cell_out, rna_cell_label):
    global LAST_RESULTS
    nc = build_nc()
    in_maps = _prep_inputs(rna_cell_out, rna_cell_label)
    res = run_bass_kernel_spmd(nc, in_maps, list(range(N_CORES)),
                               trace=TRACE, **TRACE_KW)
    LAST_RESULTS = res
    parts = [float(res.results[i]["out"][0, 0]) for i in range(N_CORES)]
    loss = np.float32(np.sum(np.array(parts, dtype=np.float64)) / B)
    return np.array([loss], dtype=np.float32)


# revision 3
# speedup vs baseline: 1.1415x; 1.1415x over previous
"""Trainium2 Bass kernel for the CellLoss problem.

loss = mean_i [ 1/(x[i, l_i] + 0.1) + sum_j x[i,j] * (x[i,j] > x[i, l_i]) ]
with x: [131072, 256] f32, l: [131072] int labels in [0, 256).

Pure data parallel across 8 NeuronCores (16384 rows each). Per core,
partition p owns rows [p*128, (p+1)*128) of the shard; tile t is the
[128, 256] block of row p*128+t per partition.

Per tile:
  gather (DVE): g[p] = sum_j (iota==l_p)*x via one fused
      scalar_tensor_tensor (stt) with a per-row sum accumulator.
  margin, engine variants cycled by PATTERN (one cycle per 16-tile chunk):
   "D": one more DVE stt, (x is_gt g) mult x with row-sum accumulator.
   "A": scalar-engine Relu(x-g) and Sign(g-x) passes writing bf16 tiles;
      the idle tensor engine then accumulates the GLOBAL sums in PSUM:
      ones^T @ relu-pairs, and g^T @ sign per tile. Using
      sum_i g_i*N+_i = (255*sum g - sum g*sign(g-x))/2 (sign(0)=0 at the
      label), the margin needs only these global sums.
   "B": scalar-engine Relu only (global sum via ones^T @ relu-pairs on
      the tensor engine); the count N+ comes from a DVE tensor_scalar
      (x is_gt g) with reduce-accumulator; count term g*N+ assembled
      per-row in the tail.
Cycle layout must be D-block, then B-block, then A-block (the tail uses
strided views of G for the B/A columns).

Tail: inv = 1/(g+0.1); per-row totals + B count terms + the A-tile
127.5*g correction; partition sum via ones-matmul; one f32 partial per
core; the host sums the 8 partials and divides by B.

bf16 is used ONLY for relu magnitudes (unbiased rounding, ~1e-6 effect),
the exact-representable sign/one/mask values, and the g matmul weights
(scales only the count term, ~1e-5 effect); g itself stays exact f32
everywhere that matters (inv term, compares).

This walrus accepts one sync wait per instruction; Tile can emit
several. _split_multi_waits() hoists extras onto Drain carriers.
"""

import numpy as np
from contextlib import ExitStack

import concourse.bass as bass
import concourse.mybir as mybir
import concourse.tile as tile
from concourse.bass_utils import run_bass_kernel_spmd

F32 = mybir.dt.float32
BF16 = mybir.dt.bfloat16

B, C = 131072, 256
N_CORES = 8
B_LOCAL = B // N_CORES          # 16384
P = 128
N_TILES = B_LOCAL // P          # 128
TILES_PER_DMA = 16              # [128, 4096] f32 = 2 MiB per DMA
N_CHUNKS = N_TILES // TILES_PER_DMA

# margin engine per tile within each 16-tile chunk; must be D*, then B*,
# then A* (tail assembly uses strided views); A count must be even
# (relu outputs pair up for the ones-matmul... A+B combined actually).
PATTERN = list("DDDDDDAAAAAAAAAA")

_NC_CACHE = {}
LAST_RESULTS = None
SPLIT_WAITS = True   # off for CoreSim (its event loop rejects bare Drains)
TRACE = False
TRACE_KW = {}


def _split_multi_waits(nc):
    for f in nc.m.functions:
        for blk in f.blocks:
            insts = list(blk.instructions)
            out = []
            changed = False
            for inst in insts:
                si = inst.sync_info
                if si is not None and si.on_wait is not None and len(si.on_wait) > 1:
                    waits = list(si.on_wait)
                    for w in waits[:-1]:
                        d = mybir.InstDrain(
                            name=nc.get_next_instruction_name(),
                            ins=[], outs=[], bass_is_fusable=False)
                        d.engine = inst.engine
                        d.sync_info = mybir.SyncInfo(on_wait=[w], on_update=[])
                        out.append(d)
                    inst.sync_info = mybir.SyncInfo(
                        on_wait=[waits[-1]], on_update=list(si.on_update or []))
                    changed = True
                out.append(inst)
            if changed:
                blk.instructions = out


def _counts():
    L = len(PATTERN)
    assert N_TILES % L == 0
    d = sum(1 for c in PATTERN if c == "D")
    b = sum(1 for c in PATTERN if c == "B")
    a = L - d - b
    assert list(PATTERN) == ["D"] * d + ["B"] * b + ["A"] * a, PATTERN
    return d, b, a


def build_nc():
    key = (tuple(PATTERN), SPLIT_WAITS)
    if key in _NC_CACHE:
        return _NC_CACHE[key]

    d_cyc, b_cyc, a_cyc = _counts()
    L = len(PATTERN)
    n_cycles = N_TILES // L
    n_relu = (b_cyc + a_cyc) * n_cycles   # relu passes total (B and A tiles)
    n_a = a_cyc * n_cycles
    n_b = b_cyc * n_cycles

    nc = bass.Bass()
    x = nc.declare_dram_parameter("x", [B_LOCAL, C], F32, isOutput=False)
    lbl = nc.declare_dram_parameter("lbl", [P, N_TILES], F32, isOutput=False)
    out = nc.declare_dram_parameter("out", [1, 1], F32, isOutput=True)

    xv = x.rearrange("(p t) c -> p (t c)", p=P, t=N_TILES)

    with tile.TileContext(nc) as tc, ExitStack() as ctx:
        singles = ctx.enter_context(tc.tile_pool(name="singles", bufs=1))
        xpool = ctx.enter_context(tc.tile_pool(name="x", bufs=3))
        scr = ctx.enter_context(tc.tile_pool(name="scr", bufs=4))
        prs = ctx.enter_context(tc.tile_pool(name="prs", bufs=4))
        psum = ctx.enter_context(tc.tile_pool(name="psum", bufs=1, space="PSUM"))

        lbl_sb = singles.tile([P, N_TILES], F32)
        nc.sync.dma_start(lbl_sb[:], lbl[:])

        iota_i = singles.tile([P, C], mybir.dt.int32)
        nc.gpsimd.iota(iota_i[:], pattern=[[1, C]], base=0, channel_multiplier=0)
        iota_f = singles.tile([P, C], F32)
        nc.vector.tensor_copy(iota_f[:], iota_i[:])

        ones = singles.tile([P, 1], F32)
        nc.vector.memset(ones[:], 1.0)

        G = singles.tile([P, N_TILES], F32)
        M = singles.tile([P, N_TILES], F32)      # D margins; B/A cols = 0
        if n_b or n_a:
            nc.vector.memset(M[:], 0.0)
            ones_bf = singles.tile([P, 1], BF16)
            nc.vector.memset(ones_bf[:], 1.0)
            NG = singles.tile([P, N_TILES], F32)   # -g (relu bias)
            ps_r = psum.tile([P, 512], F32, tag="ps_r")
        if n_b:
            CNT = singles.tile([P, N_TILES], F32)  # B-tile counts
        if n_a:
            GBF = singles.tile([P, N_TILES], BF16)  # g as bf16 matmul weight
            ps_s = [psum.tile([P, 512], F32, tag=f"ps_s{i}", name=f"ps_s{i}")
                    for i in range(2)]

        mm_r = 0
        mm_s = [0, 0]
        relu_seen = 0
        sign_seen = 0
        rpair = None
        for chunk in range(N_CHUNKS):
            xw = xpool.tile([P, TILES_PER_DMA * C], F32, name="xw")
            nc.sync.dma_start(
                xw[:],
                xv[:, chunk * TILES_PER_DMA * C:(chunk + 1) * TILES_PER_DMA * C])
            t_lo = chunk * TILES_PER_DMA

            def xb_of(kk):
                return xw[:, kk * C:(kk + 1) * C]

            # gathers for B/A tiles first so ACT/TE start early
            for kk in list(range(d_cyc, L)) + list(range(d_cyc)):
                t = t_lo + kk
                lc = lbl_sb[:, t:t + 1]
                gc = G[:, t:t + 1]
                sel = scr.tile([P, C], F32, tag="sel", name="sel")
                nc.vector.scalar_tensor_tensor(
                    out=sel[:], in0=iota_f[:], scalar=lc, in1=xb_of(kk),
                    op0=mybir.AluOpType.is_equal, op1=mybir.AluOpType.mult,
                    accum_out=gc)
                if kk == L - 1 or (d_cyc and kk == d_cyc - 1 and not (b_cyc or a_cyc)):
                    pass
                if kk == L - 1:
                    # B/A gathers done: batch-negate + bf16 weight copy
                    ba = slice(t_lo + d_cyc, t_lo + L)
                    if b_cyc or a_cyc:
                        nc.vector.tensor_scalar_mul(NG[:, ba], G[:, ba], -1.0)
                    if a_cyc:
                        aa = slice(t_lo + d_cyc + b_cyc, t_lo + L)
                        nc.vector.tensor_copy(GBF[:, aa], G[:, aa])
                if kk < d_cyc:
                    # D tile: margin immediately after its gather
                    mp = scr.tile([P, C], F32, tag="mp", name="mp")
                    nc.vector.scalar_tensor_tensor(
                        out=mp[:], in0=xb_of(kk), scalar=gc, in1=xb_of(kk),
                        op0=mybir.AluOpType.is_gt, op1=mybir.AluOpType.mult,
                        accum_out=M[:, t:t + 1])

            # B/A margin work
            for kk in range(d_cyc, L):
                t = t_lo + kk
                xb = xb_of(kk)
                gc = G[:, t:t + 1]
                # relu magnitude pass (both B and A): bf16 pairs -> TE
                u = relu_seen % 2
                if u == 0:
                    rpair = prs.tile([P, 2 * C], BF16, tag="rpair",
                                     name="rpair")
                rb = rpair[:, u * C:(u + 1) * C]
                nc.scalar.activation(
                    rb, xb, mybir.ActivationFunctionType.Relu,
                    bias=NG[:, t:t + 1], scale=1.0)
                if u == 1:
                    nc.tensor.matmul(ps_r[:1, :], ones_bf[:], rpair[:],
                                     start=(mm_r == 0),
                                     stop=(mm_r == n_relu // 2 - 1))
                    mm_r += 1
                relu_seen += 1
                if PATTERN[kk] == "B":
                    # count on DVE: (x is_gt g) reduce-add
                    cb = scr.tile([P, C], BF16, tag="cb", name="cb")
                    nc.vector.tensor_scalar(
                        out=cb[:], in0=xb, scalar1=gc, scalar2=None,
                        op0=mybir.AluOpType.is_gt, op1=mybir.AluOpType.add,
                        accum_out=CNT[:, t:t + 1])
                else:  # "A"
                    sg = scr.tile([P, C], BF16, tag="sg", name="sg")
                    nc.scalar.activation(
                        sg[:], xb, mybir.ActivationFunctionType.Sign,
                        bias=gc, scale=-1.0)
                    v = sign_seen % 2
                    nc.tensor.matmul(ps_s[v][:1, :C], GBF[:, t:t + 1], sg[:],
                                     start=(mm_s[v] == 0),
                                     stop=(mm_s[v] == (n_a + 1) // 2 - 1
                                           if v == 0 else mm_s[v] == n_a // 2 - 1))
                    mm_s[v] += 1
                    sign_seen += 1

        # ---- tail ------------------------------------------------------
        tmp = scr.tile([P, N_TILES], F32, tag="tail", name="tmp")
        nc.vector.tensor_scalar_add(tmp[:], G[:], 0.1)
        inv = scr.tile([P, N_TILES], F32, tag="tail2", name="inv")
        nc.vector.reciprocal(inv[:], tmp[:])
        tot = scr.tile([P, N_TILES], F32, tag="tail3", name="tot")
        nc.vector.tensor_tensor(out=tot[:], in0=inv[:], in1=M[:],
                                op=mybir.AluOpType.add)
        rows = singles.tile([P, 1], F32)
        nc.vector.tensor_reduce(rows[:], tot[:], axis=mybir.AxisListType.X,
                                op=mybir.AluOpType.add)
        extra_rows = []
        if n_b:
            # B count terms: g * N+ per row
            g_b = G.rearrange("p (u k) -> p u k", k=L)[:, :, d_cyc:d_cyc + b_cyc]
            cnt_b = CNT.rearrange("p (u k) -> p u k", k=L)[:, :, d_cyc:d_cyc + b_cyc]
            gcnt = singles.tile([P, n_b], F32)
            gcnt_v = gcnt.rearrange("p (u k) -> p u k", k=b_cyc)
            nc.vector.tensor_tensor(out=gcnt_v, in0=cnt_b, in1=g_b,
                                    op=mybir.AluOpType.mult)
            rows_b = singles.tile([P, 1], F32)
            nc.vector.tensor_reduce(rows_b[:], gcnt[:],
                                    axis=mybir.AxisListType.X,
                                    op=mybir.AluOpType.add)
            extra_rows.append(rows_b)
        if n_a:
            # A count correction: +127.5 * g per A column
            g_a = G.rearrange("p (u k) -> p u k", k=L)[:, :, d_cyc + b_cyc:]
            rows_ga = singles.tile([P, 1], F32)
            nc.vector.tensor_reduce(rows_ga[:], g_a,
                                    axis=mybir.AxisListType.XY,
                                    op=mybir.AluOpType.add)
            rows2 = singles.tile([P, 1], F32)
            nc.vector.tensor_scalar(out=rows2[:], in0=rows_ga[:],
                                    scalar1=127.5, scalar2=None,
                                    op0=mybir.AluOpType.mult)
            extra_rows.append(rows2)
        for ti, er in enumerate(extra_rows):
            nrows = singles.tile([P, 1], F32, name=f"rowsum{ti}")
            nc.vector.tensor_tensor(out=nrows[:], in0=rows[:], in1=er[:],
                                    op=mybir.AluOpType.add)
            rows = nrows

        ps_fin = psum.tile([P, 8], F32, tag="fin")
        nc.tensor.matmul(ps_fin[:1, :1], ones[:], rows[:])

        fin = singles.tile([1, 1], F32)
        nc.vector.tensor_copy(fin[:], ps_fin[:1, :1])
        acc_terms = [fin]
        if n_b or n_a:
            # + sum(ps_r) [relu magnitudes] - 0.5*sum(ps_s) [A count part]
            w = 512 + (512 if n_a else 0)
            cb2 = singles.tile([1, 1024], F32)
            nc.vector.tensor_copy(cb2[:, 0:512], ps_r[:1, :])
            tot1 = singles.tile([1, 1], F32)
            nc.vector.tensor_reduce(tot1[:], cb2[:, 0:512],
                                    axis=mybir.AxisListType.X,
                                    op=mybir.AluOpType.add)
            acc_terms.append(tot1)
            if n_a:
                nc.vector.tensor_copy(cb2[:, 512:768], ps_s[0][:1, :C])
                nc.vector.tensor_copy(cb2[:, 768:1024], ps_s[1][:1, :C])
                sc = singles.tile([1, 512], F32)
                nc.vector.tensor_scalar(out=sc[:], in0=cb2[:, 512:1024],
                                        scalar1=-0.5, scalar2=None,
                                        op0=mybir.AluOpType.mult)
                tot2 = singles.tile([1, 1], F32)
                nc.vector.tensor_reduce(tot2[:], sc[:],
                                        axis=mybir.AxisListType.X,
                                        op=mybir.AluOpType.add)
                acc_terms.append(tot2)
        res = acc_terms[0]
        for ti, term in enumerate(acc_terms[1:]):
            nxt = singles.tile([1, 1], F32, name=f"sumchain{ti}")
            nc.vector.tensor_tensor(out=nxt[:], in0=res[:], in1=term[:],
                                    op=mybir.AluOpType.add)
            res = nxt
        nc.sync.dma_start(out[:], res[:])

    if SPLIT_WAITS:
        _split_multi_waits(nc)
    _NC_CACHE[key] = nc
    return nc


def _prep_inputs(rna_cell_out, rna_cell_label):
    x = np.ascontiguousarray(np.asarray(rna_cell_out, dtype=np.float32))
    l = np.asarray(rna_cell_label).astype(np.int64)
    assert x.shape == (B, C) and l.shape == (B,)
    in_maps = []
    for i in range(N_CORES):
        xs = x[i * B_LOCAL:(i + 1) * B_LOCAL]
        ls = l[i * B_LOCAL:(i + 1) * B_LOCAL]
        lbl = ls.reshape(P, N_TILES).astype(np.float32)
        in_maps.append({"x": xs, "lbl": np.ascontiguousarray(lbl)})
    return in_maps


def kernel(rna_cell_out, rna_cell_label):
    global LAST_RESULTS
    nc = build_nc()
    in_maps = _prep_inputs(rna_cell_out, rna_cell_label)
    res = run_bass_kernel_spmd(nc, in_maps, list(range(N_CORES)),
                               trace=TRACE, **TRACE_KW)
    LAST_RESULTS = res
    parts = [float(res.results[i]["out"][0, 0]) for i in range(N_CORES)]
    loss = np.float32(np.sum(np.array(parts, dtype=np.float64)) / B)
    return np.array([loss], dtype=np.float32)


# revision 22
# speedup vs baseline: 1.2199x; 1.0687x over previous
"""Trainium2 Bass kernel for the CellLoss problem.

loss = mean_i [ 1/(x[i, l_i] + 0.1) + sum_j x[i,j] * (x[i,j] > x[i, l_i]) ]
with x: [131072, 256] f32, l: [131072] int labels in [0, 256).

Pure data parallel across 8 NeuronCores (16384 rows each). Per core,
partition p owns rows [p*128, (p+1)*128) of the shard; tile t is the
[128, 256] block of row p*128+t per partition.

Per tile:
  gather (DVE): g[p] = sum_j (iota==l_p)*x via one fused
      scalar_tensor_tensor (stt) with a per-row sum accumulator.
  margin, engine variants per tile (PATTERNS, one string per 16-tile
  chunk; each must be D-block then A-block):
   "D": one more DVE stt, (x is_gt g) mult x with row-sum accumulator.
   "A": scalar-engine Relu(x-g) and Sign(g-x) passes writing bf16 tiles;
      the idle tensor engine then accumulates the GLOBAL sums in PSUM:
      ones^T @ relu-pairs, and g^T @ sign per tile. Using
      sum_i g_i*N+_i = (255*sum g - sum g*sign(g-x))/2 (sign(0)=0 at the
      label), the margin needs only these global sums.
The last chunk is D-heavier so the scalar/tensor engines drain early;
PSUM accumulation is split in two segments with the first evacuated
mid-run, off the critical tail.

Tail: inv = 1/(g+0.1); per-row totals + the A-tile 127.5*g correction;
partition sum via ones-matmul; one f32 partial per core; the host sums
the 8 partials and divides by B.

bf16 is used ONLY for relu magnitudes (unbiased rounding, ~1e-6 effect),
the exact-representable sign/one values, and the g matmul weights
(scales only the count term, ~1e-5 effect); g itself stays exact f32
everywhere that matters (inv term, compares).

This walrus accepts one sync wait per instruction; Tile can emit
several. _split_multi_waits() hoists extras onto Drain carriers.
"""

import numpy as np
from contextlib import ExitStack

import concourse.bass as bass
import concourse.mybir as mybir
import concourse.tile as tile
from concourse.bass_utils import run_bass_kernel_spmd

F32 = mybir.dt.float32
BF16 = mybir.dt.bfloat16

B, C = 131072, 256
N_CORES = 8
B_LOCAL = B // N_CORES          # 16384
P = 128
N_TILES = B_LOCAL // P          # 128
TILES_PER_DMA = 16              # [128, 4096] f32 = 2 MiB per DMA
N_CHUNKS = N_TILES // TILES_PER_DMA
L = TILES_PER_DMA

# margin engine per tile, one pattern string per chunk ("D" DVE stt /
# "A" scalar engine). Each chunk must be a D-block then an A-block, and
# all chunks but the last must share one D-count (tail strided views).
PATTERNS = ["DDDDDDDAAAAAAAAA"] * 7 + ["DDDDDDDDDAAAAAAA"]
SEG_AT = 6   # chunks [0, SEG_AT) accumulate PSUM segment 0; rest segment 1
LAG = 0      # how many chunks margins trail gathers

_NC_CACHE = {}
LAST_RESULTS = None
SPLIT_WAITS = True   # off for CoreSim (its event loop rejects bare Drains)
TRACE = False
TRACE_KW = {}


def _split_multi_waits(nc):
    for f in nc.m.functions:
        for blk in f.blocks:
            insts = list(blk.instructions)
            out = []
            changed = False
            for inst in insts:
                si = inst.sync_info
                if si is not None and si.on_wait is not None and len(si.on_wait) > 1:
                    waits = list(si.on_wait)
                    for w in waits[:-1]:
                        d = mybir.InstDrain(
                            name=nc.get_next_instruction_name(),
                            ins=[], outs=[], bass_is_fusable=False)
                        d.engine = inst.engine
                        d.sync_info = mybir.SyncInfo(on_wait=[w], on_update=[])
                        out.append(d)
                    inst.sync_info = mybir.SyncInfo(
                        on_wait=[waits[-1]], on_update=list(si.on_update or []))
                    changed = True
                out.append(inst)
            if changed:
                blk.instructions = out


def _chunk_counts():
    assert len(PATTERNS) == N_CHUNKS
    ds = []
    for pat in PATTERNS:
        assert len(pat) == L
        d = sum(1 for c in pat if c == "D")
        assert list(pat) == ["D"] * d + ["A"] * (L - d), pat
        ds.append(d)
    assert len(set(ds[:-1])) == 1, "all chunks but last must share D-count"
    return ds


def build_nc():
    key = (tuple(PATTERNS), SEG_AT, LAG, SPLIT_WAITS)
    if key in _NC_CACHE:
        return _NC_CACHE[key]

    ds = _chunk_counts()
    a_per_chunk = [L - d for d in ds]
    n_a = sum(a_per_chunk)
    n_sign_seg = [sum(a_per_chunk[:SEG_AT]), sum(a_per_chunk[SEG_AT:])]
    assert n_sign_seg[0] % 2 == 0 and n_sign_seg[1] % 2 == 0

    nc = bass.Bass()
    x = nc.declare_dram_parameter("x", [B_LOCAL, C], F32, isOutput=False)
    lbl = nc.declare_dram_parameter("lbl", [P, N_TILES], F32, isOutput=False)
    out = nc.declare_dram_parameter("out", [1, 1], F32, isOutput=True)

    xv = x.rearrange("(p t) c -> p (t c)", p=P, t=N_TILES)

    with tile.TileContext(nc) as tc, ExitStack() as ctx:
        singles = ctx.enter_context(tc.tile_pool(name="singles", bufs=1))
        xpool = ctx.enter_context(tc.tile_pool(name="x", bufs=5))
        scr = ctx.enter_context(tc.tile_pool(name="scr", bufs=8))
        prs = ctx.enter_context(tc.tile_pool(name="prs", bufs=6))
        psum = ctx.enter_context(tc.tile_pool(name="psum", bufs=1, space="PSUM"))

        lbl_sb = singles.tile([P, N_TILES], F32)
        nc.sync.dma_start(lbl_sb[:], lbl[:])

        iota_i = singles.tile([P, C], mybir.dt.int32)
        nc.gpsimd.iota(iota_i[:], pattern=[[1, C]], base=0, channel_multiplier=0)
        iota_f = singles.tile([P, C], F32)
        nc.vector.tensor_copy(iota_f[:], iota_i[:])

        ones = singles.tile([P, 1], F32)
        nc.vector.memset(ones[:], 1.0)

        G = singles.tile([P, N_TILES], F32)
        M = singles.tile([P, N_TILES], F32)      # D margins; A cols = 0
        if n_a:
            nc.vector.memset(M[:], 0.0)
            ones_bf = singles.tile([P, 1], BF16)
            nc.vector.memset(ones_bf[:], 1.0)
            NG = singles.tile([P, N_TILES], F32)   # -g (relu bias)
            GPOS = singles.tile([P, N_TILES], F32)  # +g (sign bias), chunk-coarse
            GBF = singles.tile([P, N_TILES], BF16)  # g as bf16 matmul weight
            ps_r = [psum.tile([P, 512], F32, tag=f"ps_r{s}", name=f"ps_r{s}")
                    for s in range(2)]
            ps_s = [psum.tile([P, 512], F32, tag=f"ps_s{i}", name=f"ps_s{i}")
                    for i in range(4)]   # (seg, parity) -> seg*2 + parity
            cb2 = singles.tile([1, 2048], F32)     # psum evacuation buffer

        mm_r = [0, 0]
        mm_s = [0, 0, 0, 0]
        state = {"relu_seen": 0, "rpair": None}
        xw_of = {}

        def emit_gathers(chunk):
            d_c = ds[chunk]
            xw = xpool.tile([P, TILES_PER_DMA * C], F32, name="xw")
            xw_of[chunk] = xw
            base = chunk * TILES_PER_DMA * C
            if chunk == 0:
                # split the cold first chunk so compute starts early
                for lo, hi in ((0, 2), (2, 4), (4, 8), (8, 16)):
                    nc.sync.dma_start(
                        xw[:, lo * C:hi * C],
                        xv[:, base + lo * C:base + hi * C])
            else:
                nc.sync.dma_start(
                    xw[:], xv[:, base:base + TILES_PER_DMA * C])
            t_lo = chunk * TILES_PER_DMA
            korder = (list(range(L)) if chunk == 0
                      else list(range(d_c, L)) + list(range(d_c)))
            for kk in korder:
                t = t_lo + kk
                lc = lbl_sb[:, t:t + 1]
                gc = G[:, t:t + 1]
                sel = scr.tile([P, C], F32, tag="sel", name="sel")
                nc.vector.scalar_tensor_tensor(
                    out=sel[:], in0=iota_f[:], scalar=lc,
                    in1=xw[:, kk * C:(kk + 1) * C],
                    op0=mybir.AluOpType.is_equal, op1=mybir.AluOpType.mult,
                    accum_out=gc)
                if kk == L - 1 and n_a and d_c < L:
                    # A gathers done: chunk-coarse bias/weight copies so
                    # the scalar engine depends on 3 writes, not 10
                    aa = slice(t_lo + d_c, t_lo + L)
                    nc.vector.tensor_scalar_mul(NG[:, aa], G[:, aa], -1.0)
                    nc.vector.tensor_copy(GPOS[:, aa], G[:, aa])
                    nc.vector.tensor_copy(GBF[:, aa], G[:, aa])

        def emit_margins(chunk):
            d_c = ds[chunk]
            seg = 0 if chunk < SEG_AT else 1
            t_lo = chunk * TILES_PER_DMA
            xw = xw_of[chunk]
            for kk in range(d_c):
                t = t_lo + kk
                mp = scr.tile([P, C], F32, tag="mp", name="mp")
                nc.vector.scalar_tensor_tensor(
                    out=mp[:], in0=xw[:, kk * C:(kk + 1) * C],
                    scalar=G[:, t:t + 1], in1=xw[:, kk * C:(kk + 1) * C],
                    op0=mybir.AluOpType.is_gt, op1=mybir.AluOpType.mult,
                    accum_out=M[:, t:t + 1])
            for kk in range(d_c, L):
                t = t_lo + kk
                xb = xw[:, kk * C:(kk + 1) * C]
                u = state["relu_seen"] % 2
                if u == 0:
                    state["rpair"] = prs.tile([P, 2 * C], BF16, tag="rpair",
                                              name="rpair")
                rpair = state["rpair"]
                rb = rpair[:, u * C:(u + 1) * C]
                nc.scalar.activation(
                    rb, xb, mybir.ActivationFunctionType.Relu,
                    bias=NG[:, t:t + 1], scale=1.0)
                if u == 1:
                    nc.tensor.matmul(ps_r[seg][:1, :], ones_bf[:], rpair[:],
                                     start=(mm_r[seg] == 0),
                                     stop=(mm_r[seg] == n_sign_seg[seg] // 2 - 1))
                    mm_r[seg] += 1
                state["relu_seen"] += 1
                sg = scr.tile([P, C], BF16, tag="sg", name="sg")
                nc.scalar.activation(
                    sg[:], xb, mybir.ActivationFunctionType.Sign,
                    bias=GPOS[:, t:t + 1], scale=-1.0)
                v = seg * 2 + (state["relu_seen"] + 1) % 2
                nc.tensor.matmul(ps_s[v][:1, :C], GBF[:, t:t + 1], sg[:],
                                 start=(mm_s[v] == 0),
                                 stop=(mm_s[v] == n_sign_seg[seg] // 2 - 1))
                mm_s[v] += 1

        tmp = singles.tile([P, N_TILES], F32, name="tmp")
        inv = singles.tile([P, N_TILES], F32, name="inv")
        head = (N_CHUNKS - 1) * L
        for chunk in range(N_CHUNKS):
            emit_gathers(chunk)
            if chunk == N_CHUNKS - 1:
                # G[:, :head] is final: fold the inv term early
                nc.vector.tensor_scalar_add(tmp[:, :head], G[:, :head], 0.1)
                nc.vector.reciprocal(inv[:, :head], tmp[:, :head])
            if chunk >= LAG:
                emit_margins(chunk - LAG)
            if chunk == N_CHUNKS - 1 and n_a:
                # segment-0 psums are long done; evacuate while ACT works
                nc.vector.tensor_copy(cb2[:, 0:512], ps_r[0][:1, :])
                nc.vector.tensor_copy(cb2[:, 1024:1280], ps_s[0][:1, :C])
                nc.vector.tensor_copy(cb2[:, 1280:1536], ps_s[1][:1, :C])
        for chunk in range(N_CHUNKS - LAG, N_CHUNKS):
            emit_margins(chunk)

        # ---- tail ------------------------------------------------------
        nc.vector.tensor_scalar_add(tmp[:, head:], G[:, head:], 0.1)
        nc.vector.reciprocal(inv[:, head:], tmp[:, head:])
        tot = scr.tile([P, N_TILES], F32, tag="tail3", name="tot")
        nc.vector.tensor_tensor(out=tot[:], in0=inv[:], in1=M[:],
                                op=mybir.AluOpType.add)
        rows = singles.tile([P, 1], F32)
        nc.vector.tensor_reduce(rows[:], tot[:], axis=mybir.AxisListType.X,
                                op=mybir.AluOpType.add)
        if n_a:
            # A count correction: +127.5 * g per A column.
            # Chunks 0..N-2 share one D-count -> one strided view; the
            # last chunk gets its own flat slice.
            d0 = ds[0]
            n_head = N_CHUNKS - 1
            rga = singles.tile([P, 2], F32)
            nc.vector.memset(rga[:], 0.0)
            if d0 < L:
                g_head = (G[:, :n_head * L]
                          .rearrange("p (u k) -> p u k", k=L)[:, :, d0:])
                nc.vector.tensor_reduce(rga[:, 0:1], g_head,
                                        axis=mybir.AxisListType.XY,
                                        op=mybir.AluOpType.add)
            if ds[-1] < L:
                g_last = G[:, (n_head * L) + ds[-1]:]
                nc.vector.tensor_reduce(rga[:, 1:2], g_last,
                                        axis=mybir.AxisListType.X,
                                        op=mybir.AluOpType.add)
            rows_ga = singles.tile([P, 1], F32)
            nc.vector.tensor_reduce(rows_ga[:], rga[:],
                                    axis=mybir.AxisListType.X,
                                    op=mybir.AluOpType.add)
            rows2 = singles.tile([P, 1], F32)
            nc.vector.tensor_scalar(out=rows2[:], in0=rows_ga[:],
                                    scalar1=127.5, scalar2=None,
                                    op0=mybir.AluOpType.mult)
            rows3 = singles.tile([P, 1], F32)
            nc.vector.tensor_tensor(out=rows3[:], in0=rows[:], in1=rows2[:],
                                    op=mybir.AluOpType.add)
            rows = rows3

        ps_fin = psum.tile([P, 8], F32, tag="fin")
        nc.tensor.matmul(ps_fin[:1, :1], ones[:], rows[:])

        fin = singles.tile([1, 1], F32)
        nc.vector.tensor_copy(fin[:], ps_fin[:1, :1])
        acc_terms = [fin]
        if n_a:
            # + sum(ps_r) [relu magnitudes] - 0.5*sum(ps_s) [count part]
            nc.vector.tensor_copy(cb2[:, 512:1024], ps_r[1][:1, :])
            tot1 = singles.tile([1, 1], F32)
            nc.vector.tensor_reduce(tot1[:], cb2[:, 0:1024],
                                    axis=mybir.AxisListType.X,
                                    op=mybir.AluOpType.add)
            acc_terms.append(tot1)
            nc.vector.tensor_copy(cb2[:, 1536:1792], ps_s[2][:1, :C])
            nc.vector.tensor_copy(cb2[:, 1792:2048], ps_s[3][:1, :C])
            sc = singles.tile([1, 1024], F32)
            nc.vector.tensor_scalar(out=sc[:], in0=cb2[:, 1024:2048],
                                    scalar1=-0.5, scalar2=None,
                                    op0=mybir.AluOpType.mult)
            tot2 = singles.tile([1, 1], F32)
            nc.vector.tensor_reduce(tot2[:], sc[:],
                                    axis=mybir.AxisListType.X,
                                    op=mybir.AluOpType.add)
            acc_terms.append(tot2)
        res = acc_terms[0]
        for ti, term in enumerate(acc_terms[1:]):
            nxt = singles.tile([1, 1], F32, name=f"sumchain{ti}")
            nc.vector.tensor_tensor(out=nxt[:], in0=res[:], in1=term[:],
                                    op=mybir.AluOpType.add)
            res = nxt
        nc.sync.dma_start(out[:], res[:])

    if SPLIT_WAITS:
        _split_multi_waits(nc)
    _NC_CACHE[key] = nc
    return nc


def _prep_inputs(rna_cell_out, rna_cell_label):
    x = np.ascontiguousarray(np.asarray(rna_cell_out, dtype=np.float32))
    l = np.asarray(rna_cell_label).astype(np.int64)
    assert x.shape == (B, C) and l.shape == (B,)
    in_maps = []
    for i in range(N_CORES):
        xs = x[i * B_LOCAL:(i + 1) * B_LOCAL]
        ls = l[i * B_LOCAL:(i + 1) * B_LOCAL]
        lbl = ls.reshape(P, N_TILES).astype(np.float32)
        in_maps.append({"x": xs, "lbl": np.ascontiguousarray(lbl)})
    return in_maps


def kernel(rna_cell_out, rna_cell_label):
    global LAST_RESULTS
    nc = build_nc()
    in_maps = _prep_inputs(rna_cell_out, rna_cell_label)
    res = run_bass_kernel_spmd(nc, in_maps, list(range(N_CORES)),
                               trace=TRACE, **TRACE_KW)
    LAST_RESULTS = res
    parts = [float(res.results[i]["out"][0, 0]) for i in range(N_CORES)]
    loss = np.float32(np.sum(np.array(parts, dtype=np.float64)) / B)
    return np.array([loss], dtype=np.float32)


# revision 30
# speedup vs baseline: 1.2384x; 1.0152x over previous
"""Trainium2 Bass kernel for the CellLoss problem.

loss = mean_i [ 1/(x[i, l_i] + 0.1) + sum_j x[i,j] * (x[i,j] > x[i, l_i]) ]
with x: [131072, 256] f32, l: [131072] int labels in [0, 256).

Pure data parallel across 8 NeuronCores (16384 rows each). Per core,
partition p owns rows [p*128, (p+1)*128) of the shard; tile t is the
[128, 256] block of row p*128+t per partition.

Per tile:
  gather (DVE): g[p] = sum_j (iota==l_p)*x via one fused
      scalar_tensor_tensor (stt) with a per-row sum accumulator.
  margin, engine variants per tile (PATTERNS, one string per 16-tile
  chunk; each must be D-block then A-block):
   "D": one more DVE stt, (x is_gt g) mult x with row-sum accumulator.
   "A": scalar-engine Relu(x-g) and Sign(g-x) passes writing bf16 tiles;
      the idle tensor engine then accumulates the GLOBAL sums in PSUM:
      ones^T @ relu-pairs, and g^T @ sign per tile. Using
      sum_i g_i*N+_i = (255*sum g - sum g*sign(g-x))/2 (sign(0)=0 at the
      label), the margin needs only these global sums.
The last chunk is D-heavier so the scalar/tensor engines drain early;
PSUM accumulation is split in two segments with the first evacuated
mid-run, off the critical tail.

Tail: inv = 1/(g+0.1); per-row totals + the A-tile 127.5*g correction;
partition sum via ones-matmul; one f32 partial per core; the host sums
the 8 partials and divides by B.

bf16 is used ONLY for relu magnitudes (unbiased rounding, ~1e-6 effect),
the exact-representable sign/one values, and the g matmul weights
(scales only the count term, ~1e-5 effect); g itself stays exact f32
everywhere that matters (inv term, compares).

This walrus accepts one sync wait per instruction; Tile can emit
several. _split_multi_waits() hoists extras onto Drain carriers.
"""

import numpy as np
from contextlib import ExitStack

import concourse.bass as bass
import concourse.mybir as mybir
import concourse.tile as tile
from concourse.bass_utils import run_bass_kernel_spmd

F32 = mybir.dt.float32
BF16 = mybir.dt.bfloat16

B, C = 131072, 256
N_CORES = 8
B_LOCAL = B // N_CORES          # 16384
P = 128
N_TILES = B_LOCAL // P          # 128
TILES_PER_DMA = 16              # [128, 4096] f32 = 2 MiB per DMA
N_CHUNKS = N_TILES // TILES_PER_DMA
L = TILES_PER_DMA

# margin engine per tile, one pattern string per chunk ("D" DVE stt /
# "A" scalar engine). Each chunk must be a D-block then an A-block, and
# all chunks but the last must share one D-count (tail strided views).
PATTERNS = ["DDDDDDDAAAAAAAAA"] * 7 + ["DDDDDDDDDAAAAAAA"]
SEG_AT = 6   # chunks [0, SEG_AT) accumulate PSUM segment 0; rest segment 1
LAG = 0      # how many chunks margins trail gathers

_NC_CACHE = {}
LAST_RESULTS = None
SPLIT_WAITS = True   # off for CoreSim (its event loop rejects bare Drains)
TRACE = False
TRACE_KW = {}


def _split_multi_waits(nc):
    for f in nc.m.functions:
        for blk in f.blocks:
            insts = list(blk.instructions)
            out = []
            changed = False
            for inst in insts:
                si = inst.sync_info
                if si is not None and si.on_wait is not None and len(si.on_wait) > 1:
                    waits = list(si.on_wait)
                    for w in waits[:-1]:
                        d = mybir.InstDrain(
                            name=nc.get_next_instruction_name(),
                            ins=[], outs=[], bass_is_fusable=False)
                        d.engine = inst.engine
                        d.sync_info = mybir.SyncInfo(on_wait=[w], on_update=[])
                        out.append(d)
                    inst.sync_info = mybir.SyncInfo(
                        on_wait=[waits[-1]], on_update=list(si.on_update or []))
                    changed = True
                out.append(inst)
            if changed:
                blk.instructions = out


def _chunk_counts():
    assert len(PATTERNS) == N_CHUNKS
    ds = []
    for pat in PATTERNS:
        assert len(pat) == L
        d = sum(1 for c in pat if c == "D")
        assert list(pat) == ["D"] * d + ["A"] * (L - d), pat
        ds.append(d)
    assert len(set(ds[:-1])) == 1, "all chunks but last must share D-count"
    return ds


def build_nc():
    key = (tuple(PATTERNS), SEG_AT, LAG, SPLIT_WAITS)
    if key in _NC_CACHE:
        return _NC_CACHE[key]

    ds = _chunk_counts()
    a_per_chunk = [L - d for d in ds]
    n_a = sum(a_per_chunk)
    n_sign_seg = [sum(a_per_chunk[:SEG_AT]), sum(a_per_chunk[SEG_AT:])]
    assert n_sign_seg[0] % 2 == 0 and n_sign_seg[1] % 2 == 0

    nc = bass.Bass()
    x = nc.declare_dram_parameter("x", [B_LOCAL, C], F32, isOutput=False)
    lbl = nc.declare_dram_parameter("lbl", [P, N_TILES], F32, isOutput=False)
    out = nc.declare_dram_parameter("out", [1, 1], F32, isOutput=True)

    xv = x.rearrange("(p t) c -> p (t c)", p=P, t=N_TILES)

    with tile.TileContext(nc) as tc, ExitStack() as ctx:
        singles = ctx.enter_context(tc.tile_pool(name="singles", bufs=1))
        xpool = ctx.enter_context(tc.tile_pool(name="x", bufs=5))
        scr = ctx.enter_context(tc.tile_pool(name="scr", bufs=8))
        prs = ctx.enter_context(tc.tile_pool(name="prs", bufs=6))
        psum = ctx.enter_context(tc.tile_pool(name="psum", bufs=1, space="PSUM"))

        lbl_sb = singles.tile([P, N_TILES], F32)
        nc.sync.dma_start(lbl_sb[:], lbl[:])

        iota_i = singles.tile([P, C], mybir.dt.int32)
        nc.gpsimd.iota(iota_i[:], pattern=[[1, C]], base=0, channel_multiplier=0)
        iota_f = singles.tile([P, C], F32)
        nc.vector.tensor_copy(iota_f[:], iota_i[:])

        ones = singles.tile([P, 1], F32)
        nc.vector.memset(ones[:], 1.0)

        G = singles.tile([P, N_TILES], F32)
        M = singles.tile([P, N_TILES], F32)      # D margins; A cols = 0
        if n_a:
            nc.vector.memset(M[:], 0.0)
            ones_bf = singles.tile([P, 1], BF16)
            nc.vector.memset(ones_bf[:], 1.0)
            NG = singles.tile([P, N_TILES], F32)   # -g (relu bias)
            GPOS = singles.tile([P, N_TILES], F32)  # +g (sign bias), chunk-coarse
            GBF = singles.tile([P, N_TILES], BF16)  # g as bf16 matmul weight
            ps_r = [psum.tile([P, 512], F32, tag=f"ps_r{s}", name=f"ps_r{s}")
                    for s in range(2)]
            ps_s = [psum.tile([P, 512], F32, tag=f"ps_s{i}", name=f"ps_s{i}")
                    for i in range(4)]   # (seg, parity) -> seg*2 + parity
            cb2 = singles.tile([1, 2048], F32)     # psum evacuation buffer

        mm_r = [0, 0]
        mm_s = [0, 0, 0, 0]
        state = {"relu_seen": 0, "rpair": None}
        xw_of = {}

        def emit_gathers(chunk):
            d_c = ds[chunk]
            xw = xpool.tile([P, TILES_PER_DMA * C], F32, name="xw")
            xw_of[chunk] = xw
            base = chunk * TILES_PER_DMA * C
            if chunk == 0:
                # split the cold first chunk so compute starts early
                for lo, hi in ((0, 2), (2, 4), (4, 8), (8, 16)):
                    nc.sync.dma_start(
                        xw[:, lo * C:hi * C],
                        xv[:, base + lo * C:base + hi * C])
            else:
                nc.sync.dma_start(
                    xw[:], xv[:, base:base + TILES_PER_DMA * C])
            t_lo = chunk * TILES_PER_DMA
            korder = (list(range(L)) if chunk == 0
                      else list(range(d_c, L)) + list(range(d_c)))
            for kk in korder:
                t = t_lo + kk
                lc = lbl_sb[:, t:t + 1]
                gc = G[:, t:t + 1]
                sel = scr.tile([P, C], F32, tag="sel", name="sel")
                nc.vector.scalar_tensor_tensor(
                    out=sel[:], in0=iota_f[:], scalar=lc,
                    in1=xw[:, kk * C:(kk + 1) * C],
                    op0=mybir.AluOpType.is_equal, op1=mybir.AluOpType.mult,
                    accum_out=gc)
                if kk == L - 1 and n_a and d_c < L:
                    # A gathers done: chunk-coarse bias/weight copies so
                    # the scalar engine depends on 3 writes, not 10
                    aa = slice(t_lo + d_c, t_lo + L)
                    nc.vector.tensor_scalar_mul(NG[:, aa], G[:, aa], -1.0)
                    nc.vector.tensor_copy(GPOS[:, aa], G[:, aa])
                    nc.vector.tensor_copy(GBF[:, aa], G[:, aa])

        def emit_margins(chunk):
            d_c = ds[chunk]
            seg = 0 if chunk < SEG_AT else 1
            t_lo = chunk * TILES_PER_DMA
            xw = xw_of[chunk]
            for kk in range(d_c):
                t = t_lo + kk
                mp = scr.tile([P, C], F32, tag="mp", name="mp")
                nc.vector.scalar_tensor_tensor(
                    out=mp[:], in0=xw[:, kk * C:(kk + 1) * C],
                    scalar=G[:, t:t + 1], in1=xw[:, kk * C:(kk + 1) * C],
                    op0=mybir.AluOpType.is_gt, op1=mybir.AluOpType.mult,
                    accum_out=M[:, t:t + 1])
            for kk in range(d_c, L):
                t = t_lo + kk
                xb = xw[:, kk * C:(kk + 1) * C]
                u = state["relu_seen"] % 2
                if u == 0:
                    state["rpair"] = prs.tile([P, 2 * C], BF16, tag="rpair",
                                              name="rpair")
                rpair = state["rpair"]
                rb = rpair[:, u * C:(u + 1) * C]
                nc.scalar.activation(
                    rb, xb, mybir.ActivationFunctionType.Relu,
                    bias=NG[:, t:t + 1], scale=1.0)
                if u == 1:
                    nc.tensor.matmul(ps_r[seg][:1, :], ones_bf[:], rpair[:],
                                     start=(mm_r[seg] == 0),
                                     stop=(mm_r[seg] == n_sign_seg[seg] // 2 - 1))
                    mm_r[seg] += 1
                state["relu_seen"] += 1
                sg = scr.tile([P, C], BF16, tag="sg", name="sg")
                nc.scalar.activation(
                    sg[:], xb, mybir.ActivationFunctionType.Sign,
                    bias=GPOS[:, t:t + 1], scale=-1.0)
                v = seg * 2 + (state["relu_seen"] + 1) % 2
                nc.tensor.matmul(ps_s[v][:1, :C], GBF[:, t:t + 1], sg[:],
                                 start=(mm_s[v] == 0),
                                 stop=(mm_s[v] == n_sign_seg[seg] // 2 - 1))
                mm_s[v] += 1

        tmp = singles.tile([P, N_TILES], F32, name="tmp")
        inv = singles.tile([P, N_TILES], F32, name="inv")
        head = (N_CHUNKS - 1) * L
        for chunk in range(N_CHUNKS):
            emit_gathers(chunk)
            if chunk == N_CHUNKS - 1:
                # G[:, :head] is final: fold the inv term early
                nc.vector.tensor_scalar_add(tmp[:, :head], G[:, :head], 0.1)
                nc.vector.reciprocal(inv[:, :head], tmp[:, :head])
            if chunk >= LAG:
                emit_margins(chunk - LAG)
            if chunk == N_CHUNKS - 1 and n_a:
                # segment-0 psums are long done; evacuate while ACT works
                nc.vector.tensor_copy(cb2[:, 0:512], ps_r[0][:1, :])
                nc.vector.tensor_copy(cb2[:, 1024:1280], ps_s[0][:1, :C])
                nc.vector.tensor_copy(cb2[:, 1280:1536], ps_s[1][:1, :C])
        for chunk in range(N_CHUNKS - LAG, N_CHUNKS):
            emit_margins(chunk)

        # ---- tail ------------------------------------------------------
        nc.vector.tensor_scalar_add(tmp[:, head:], G[:, head:], 0.1)
        nc.vector.reciprocal(inv[:, head:], tmp[:, head:])
        tot = scr.tile([P, N_TILES], F32, tag="tail3", name="tot")
        nc.vector.tensor_tensor(out=tot[:], in0=inv[:], in1=M[:],
                                op=mybir.AluOpType.add)
        rows = singles.tile([P, 1], F32)
        nc.vector.tensor_reduce(rows[:], tot[:], axis=mybir.AxisListType.X,
                                op=mybir.AluOpType.add)
        if n_a:
            # A count correction: +127.5 * g per A column.
            # Chunks 0..N-2 share one D-count -> one strided view; the
            # last chunk gets its own flat slice.
            d0 = ds[0]
            n_head = N_CHUNKS - 1
            rga = singles.tile([P, 2], F32)
            nc.vector.memset(rga[:], 0.0)
            if d0 < L:
                g_head = (G[:, :n_head * L]
                          .rearrange("p (u k) -> p u k", k=L)[:, :, d0:])
                nc.vector.tensor_reduce(rga[:, 0:1], g_head,
                                        axis=mybir.AxisListType.XY,
                                        op=mybir.AluOpType.add)
            if ds[-1] < L:
                g_last = G[:, (n_head * L) + ds[-1]:]
                nc.vector.tensor_reduce(rga[:, 1:2], g_last,
                                        axis=mybir.AxisListType.X,
                                        op=mybir.AluOpType.add)
            rows_ga = singles.tile([P, 1], F32)
            nc.vector.tensor_reduce(rows_ga[:], rga[:],
                                    axis=mybir.AxisListType.X,
                                    op=mybir.AluOpType.add)
            rows2 = singles.tile([P, 1], F32)
            nc.vector.tensor_scalar(out=rows2[:], in0=rows_ga[:],
                                    scalar1=127.5, scalar2=None,
                                    op0=mybir.AluOpType.mult)
            rows3 = singles.tile([P, 1], F32)
            nc.vector.tensor_tensor(out=rows3[:], in0=rows[:], in1=rows2[:],
                                    op=mybir.AluOpType.add)
            rows = rows3

        ps_fin = psum.tile([P, 8], F32, tag="fin")
        nc.tensor.matmul(ps_fin[:1, :1], ones[:], rows[:])

        fin = singles.tile([1, 1], F32)
        nc.vector.tensor_copy(fin[:], ps_fin[:1, :1])
        acc_terms = [fin]
        if n_a:
            # + sum(ps_r) [relu magnitudes] - 0.5*sum(ps_s) [count part]
            nc.vector.tensor_copy(cb2[:, 512:1024], ps_r[1][:1, :])
            tot1 = singles.tile([1, 1], F32)
            nc.vector.tensor_reduce(tot1[:], cb2[:, 0:1024],
                                    axis=mybir.AxisListType.X,
                                    op=mybir.AluOpType.add)
            acc_terms.append(tot1)
            nc.vector.tensor_copy(cb2[:, 1536:1792], ps_s[2][:1, :C])
            nc.vector.tensor_copy(cb2[:, 1792:2048], ps_s[3][:1, :C])
            sc = singles.tile([1, 1024], F32)
            nc.vector.tensor_scalar(out=sc[:], in0=cb2[:, 1024:2048],
                                    scalar1=-0.5, scalar2=None,
                                    op0=mybir.AluOpType.mult)
            tot2 = singles.tile([1, 1], F32)
            nc.vector.tensor_reduce(tot2[:], sc[:],
                                    axis=mybir.AxisListType.X,
                                    op=mybir.AluOpType.add)
            acc_terms.append(tot2)
        res = acc_terms[0]
        for ti, term in enumerate(acc_terms[1:]):
            nxt = singles.tile([1, 1], F32, name=f"sumchain{ti}")
            nc.vector.tensor_tensor(out=nxt[:], in0=res[:], in1=term[:],
                                    op=mybir.AluOpType.add)
            res = nxt
        nc.sync.dma_start(out[:], res[:])

    if SPLIT_WAITS:
        _split_multi_waits(nc)
    _NC_CACHE[key] = nc
    return nc


def _prep_inputs(rna_cell_out, rna_cell_label):
    x = np.ascontiguousarray(np.asarray(rna_cell_out, dtype=np.float32))
    l = np.asarray(rna_cell_label).astype(np.int64)
    assert x.shape == (B, C) and l.shape == (B,)
    in_maps = []
    for i in range(N_CORES):
        xs = x[i * B_LOCAL:(i + 1) * B_LOCAL]
        ls = l[i * B_LOCAL:(i + 1) * B_LOCAL]
        lbl = ls.reshape(P, N_TILES).astype(np.float32)
        in_maps.append({"x": xs, "lbl": np.ascontiguousarray(lbl)})
    return in_maps


def kernel(rna_cell_out, rna_cell_label):
    global LAST_RESULTS
    nc = build_nc()
    in_maps = _prep_inputs(rna_cell_out, rna_cell_label)
    res = run_bass_kernel_spmd(nc, in_maps, list(range(N_CORES)),
                               trace=TRACE, **TRACE_KW)
    LAST_RESULTS = res
    parts = [float(res.results[i]["out"][0, 0]) for i in range(N_CORES)]
    loss = np.float32(np.sum(np.array(parts, dtype=np.float64)) / B)
    return np.array([loss], dtype=np.float32)
